# revision 9
# baseline (speedup 1.0000x reference)
"""Trainium2 Bass kernel for nn_Actor (ragged spline actor head).

Strategy: pure data-parallel over batch B=16384 across 8 NeuronCores
(2048 rows/core). The device kernel computes the MLP trunk:
  x=[latent,intent,1] @ W1p -> LayerNorm -> affine -> ELU -> @W2 -> @Wp
producing raw (B, 248) (biases b2/bp are folded into a constant vector
added on host: raw += b2@Wp + bp). Activations are shipped
pre-transposed (feature-on-partition) so layer-1 needs no on-device
transpose; layers 2/3 transpose activations on the TensorEngine.

fp32 matmuls lower to a self-loading LDWEIGHTS whose ISA wait table
holds a single sync wait, so every SBUF operand feeding the PE is
produced by a final ACT (ScalarE) instruction and every PSUM slot's
last consumer is ACT — all PE waits then collapse onto the one ACT
semaphore.

The ragged Catmull-Rom spline stage is LINEAR in the knots for each of
the 16 possible middle-knot mask patterns (boundary knots always
active), so it reduces to traj[n] = S[p(n)] @ combined[n] with a
precomputed (16, 40, 6) selection/basis table S; the cheap geometry +
selection runs vectorized on host.
"""

import sys

sys.path.insert(0, "/opt/trn_rl_repo")

import numpy as np

B, M, K, RES = 16384, 8, 6, 40
LATENT, INTENT, H = 1024, 3, 256
OUTC = M * (1 + K * 5)  # 248
NCORES = 8
BS = B // NCORES  # 2048 rows per core
NT = BS // 128  # 16 row-tiles per core
FC = 9  # feature chunks: 1024 latent + 3 intent + 1 ones + pad -> 1152
FPAD = FC * 128
PPS = {k: max(4, int(np.ceil(RES / (k - 1)))) for k in range(2, K + 1)}
MAX_T = 40

_CACHE = {}


# ---------------------------------------------------------------- spline table
def _catmull_basis(k):
    """C_k (MAX_T, k): linear map active-knots -> trajectory, matching
    reference._catmull_fixed_k (reflected end padding, last-point pad)."""
    pts = np.eye(k, dtype=np.float64)[:, :, None]  # (k_basis, k, 1)
    p0 = 2 * pts[:, :1] - pts[:, 1:2]
    pn = 2 * pts[:, -1:] - pts[:, -2:-1]
    pad = np.concatenate([p0, pts, pn], axis=1)  # (k, k+2, 1)
    t = np.linspace(0.0, 1.0, PPS[k])[:-1][None, None, :, None]
    t2, t3 = t * t, t * t * t
    P0, P1 = pad[:, 0 : k - 1, None], pad[:, 1:k, None]
    P2, P3 = pad[:, 2 : k + 1, None], pad[:, 3 : k + 2, None]
    seg = 0.5 * (
        2 * P1
        + (-P0 + P2) * t
        + (2 * P0 - 5 * P1 + 4 * P2 - P3) * t2
        + (-P0 + 3 * P1 - 3 * P2 + P3) * t3
    )  # (k, k-1, pps-1, 1)
    traj = np.concatenate([seg.reshape(k, -1, 1), pts[:, -1:]], axis=1)
    padn = MAX_T - traj.shape[1]
    if padn > 0:
        traj = np.concatenate([traj, np.repeat(traj[:, -1:], padn, axis=1)], axis=1)
    return traj[..., 0].T  # (MAX_T, k)


def _sel_table():
    """S (16, MAX_T, 6): traj = S[pattern] @ knots6 where pattern bits are
    the activity of middle knots 1..4 (knot 0 and 5 always active)."""
    S = np.zeros((16, MAX_T, K), dtype=np.float64)
    for p in range(16):
        act = [0] + [i + 1 for i in range(4) if (p >> i) & 1] + [5]
        C = _catmull_basis(len(act))
        for i, j in enumerate(act):
            S[p, :, j] = C[:, i]
    return S.astype(np.float32)


_S16 = _sel_table()


# ---------------------------------------------------------------- bass builder
def _build_bass():
    from concourse import bacc
    import concourse.mybir as mybir
    from concourse.tile import TileContext
    from concourse.masks import make_identity

    AF = mybir.ActivationFunctionType
    AL = mybir.AluOpType
    f32 = mybir.dt.float32

    nc = bacc.Bacc()
    xt = nc.dram_tensor("xt", [NT, 128, FC, 128], f32, kind="ExternalInput")
    w1 = nc.dram_tensor("w1", [128, FC, H], f32, kind="ExternalInput")
    w2 = nc.dram_tensor("w2", [128, 2, H], f32, kind="ExternalInput")
    wp = nc.dram_tensor("wp", [128, 2, OUTC], f32, kind="ExternalInput")
    gb = nc.dram_tensor("gb", [128, H], f32, kind="ExternalInput")
    bb = nc.dram_tensor("bb", [128, H], f32, kind="ExternalInput")
    raw = nc.dram_tensor("raw", [NT, 128, OUTC], f32, kind="ExternalOutput")

    with TileContext(nc) as tc:
        with (
            tc.tile_pool(name="consts", bufs=1) as cp,
            tc.tile_pool(name="work", bufs=3) as wk,
            tc.tile_pool(name="psum", bufs=2, space="PSUM") as pp,
            tc.tile_pool(name="psumt", bufs=2, space="PSUM") as pt,
        ):
            # Weights: DMA to a staging tile, then ACT-copy to the tile the
            # PE reads, so PE waits collapse onto the ACT semaphore.
            w1s = cp.tile([128, FC, H], f32)
            nc.sync.dma_start(w1s[:], w1[:])
            w2s = cp.tile([128, 2, H], f32)
            nc.sync.dma_start(w2s[:], w2[:])
            wps = cp.tile([128, 2, OUTC], f32)
            nc.sync.dma_start(wps[:], wp[:])
            ident = cp.tile([128, 128], f32)
            make_identity(nc, ident[:])

            gbs = cp.tile([128, H], f32)
            nc.sync.dma_start(gbs[:], gb[:])
            bbs = cp.tile([128, H], f32)
            nc.sync.dma_start(bbs[:], bb[:])
            neg1 = cp.tile([128, 1], f32)
            nc.vector.memset(neg1[:], -1.0)

            for t in range(NT):
                xsb = wk.tile([128, FC, 128], f32, tag="xsb")
                nc.sync.dma_start(xsb[:], xt[t])

                ph = pp.tile([128, H], f32, tag="ph")
                for c in range(FC):
                    nc.tensor.matmul(
                        ph[:], xsb[:, c, :], w1s[:, c, :],
                        start=(c == 0), stop=(c == FC - 1),
                    )

                # LayerNorm over H (free dim): stats via ACT accumulate.
                # ph's readers are both ACT so the next ph matmul has 1 wait.
                hsb = wk.tile([128, H], f32, tag="hsb")
                s1 = wk.tile([128, 1], f32, tag="s1")
                nc.scalar.activation(hsb[:], ph[:], AF.Identity, accum_out=s1[:])
                h2 = wk.tile([128, H], f32, tag="h2")
                s2 = wk.tile([128, 1], f32, tag="s2")
                nc.scalar.activation(h2[:], ph[:], AF.Square, accum_out=s2[:])
                mu = wk.tile([128, 1], f32, tag="mu")
                nc.scalar.mul(mu[:], s1[:], 1.0 / H)
                ex2 = wk.tile([128, 1], f32, tag="ex2")
                nc.scalar.mul(ex2[:], s2[:], 1.0 / H)
                mu2 = wk.tile([128, 1], f32, tag="mu2")
                nc.vector.tensor_mul(mu2[:], mu[:], mu[:])
                var = wk.tile([128, 1], f32, tag="var")
                nc.vector.tensor_sub(var[:], ex2[:], mu2[:])
                nc.vector.tensor_scalar_add(var[:], var[:], 1e-5)
                std = wk.tile([128, 1], f32, tag="std")
                nc.scalar.activation(std[:], var[:], AF.Sqrt)
                inv = wk.tile([128, 1], f32, tag="inv")
                nc.vector.reciprocal(inv[:], std[:])
                hn = wk.tile([128, H], f32, tag="hn")
                nc.vector.tensor_scalar(
                    hn[:], hsb[:], mu[:], inv[:], AL.subtract, AL.mult
                )
                nc.vector.tensor_mul(hn[:], hn[:], gbs[:])
                nc.vector.tensor_add(hn[:], hn[:], bbs[:])

                # ELU = max(x,0) + exp(min(x,0)) - 1; final writer is ACT
                tmin = wk.tile([128, H], f32, tag="tmin")
                nc.vector.tensor_scalar_min(tmin[:], hn[:], 0.0)
                ee = wk.tile([128, H], f32, tag="ee")
                nc.scalar.activation(ee[:], tmin[:], AF.Exp)
                q = wk.tile([128, H], f32, tag="q")
                nc.vector.tensor_scalar_max(q[:], hn[:], 0.0)
                nc.vector.tensor_add(q[:], q[:], ee[:])
                el = wk.tile([128, H], f32, tag="el")
                nc.scalar.activation(el[:], q[:], AF.Identity, bias=neg1[:])

                # transpose elu -> feature-on-partition
                eT = wk.tile([128, 2, 128], f32, tag="eT")
                for c in range(2):
                    ptt = pt.tile([128, 128], f32, tag="tp")
                    nc.tensor.transpose(
                        ptt[:], el[:, c * 128 : (c + 1) * 128], ident[:]
                    )
                    nc.scalar.copy(eT[:, c, :], ptt[:])

                ps = pp.tile([128, H], f32, tag="ps")
                for c in range(2):
                    nc.tensor.matmul(
                        ps[:], eT[:, c, :], w2s[:, c, :],
                        start=(c == 0), stop=(c == 1),
                    )
                ssb = wk.tile([128, H], f32, tag="ssb")
                nc.scalar.copy(ssb[:], ps[:])

                sT = wk.tile([128, 2, 128], f32, tag="sT")
                for c in range(2):
                    ptt = pt.tile([128, 128], f32, tag="tp")
                    nc.tensor.transpose(
                        ptt[:], ssb[:, c * 128 : (c + 1) * 128], ident[:]
                    )
                    nc.scalar.copy(sT[:, c, :], ptt[:])

                pr = pp.tile([128, OUTC], f32, tag="pr")
                for c in range(2):
                    nc.tensor.matmul(
                        pr[:], sT[:, c, :], wps[:, c, :],
                        start=(c == 0), stop=(c == 1),
                    )
                rsb = wk.tile([128, OUTC], f32, tag="rsb")
                nc.scalar.copy(rsb[:], pr[:])
                nc.sync.dma_start(raw[t], rsb[:])
    nc.compile()
    return nc


# ---------------------------------------------------------------- host helpers
def _prep_inputs(latent, intent, W1, b1, ln_g, ln_b):
    """Build per-core in_maps (weights replicated, x sharded+pretransposed)."""
    xp = np.zeros((B, FPAD), dtype=np.float32)
    xp[:, :LATENT] = latent
    xp[:, LATENT : LATENT + INTENT] = intent
    xp[:, LATENT + INTENT] = 1.0  # ones column -> b1 via matmul

    W1p = np.zeros((FPAD, H), dtype=np.float32)
    W1p[: LATENT + INTENT] = W1
    W1p[LATENT + INTENT] = b1
    w1m = np.ascontiguousarray(W1p.reshape(FC, 128, H).transpose(1, 0, 2))
    gbm = np.ascontiguousarray(np.broadcast_to(ln_g, (128, H)).astype(np.float32))
    bbm = np.ascontiguousarray(np.broadcast_to(ln_b, (128, H)).astype(np.float32))

    in_maps = []
    for c in range(NCORES):
        xs = xp[c * BS : (c + 1) * BS]  # (2048, 1152)
        # device tile layout: [t, feat_in_chunk, chunk, batch_col]
        xtc = np.ascontiguousarray(
            xs.reshape(NT, 128, FC, 128).transpose(0, 3, 2, 1)
        )
        in_maps.append(
            {
                "xt": xtc,
                "w1": w1m,
                "w2": _CACHE["w2m"],
                "wp": _CACHE["wpm"],
                "gb": gbm,
                "bb": bbm,
            }
        )
    return in_maps


def _postprocess(raw, intent, previous_velocity):
    """Geometry + ragged-spline stage (vectorized numpy), matching reference."""
    raw = raw.reshape(B, M, 1 + K * 5)
    logits = np.ascontiguousarray(raw[:, :, 0])
    geo = raw[:, :, 1:].reshape(B, M, K, 5)
    knot_steps = np.tanh(geo[..., :3]) * 2.0
    sigmas_raw = np.logaddexp(0.0, geo[..., 3:4]).astype(np.float32) + 0.1
    mask_raw = 1.0 / (1.0 + np.exp(-geo[..., 4]))
    ones = np.ones((B, M, 1), mask_raw.dtype)
    knot_mask = np.concatenate([ones, mask_raw[:, :, 1:-1], ones], axis=-1)
    knots_rel = np.cumsum(knot_steps, axis=2)
    knots_rel = knots_rel - knots_rel[:, :, 0:1, :]
    pv = previous_velocity
    pvn = pv / (np.linalg.norm(pv, axis=-1, keepdims=True) + 1e-6)
    fs = knots_rel[:, :, 1:2, :]
    fsn = fs / (np.linalg.norm(fs, axis=-1, keepdims=True) + 1e-6)
    ad = 0.8 * pvn[:, None, None, :] + 0.2 * fsn
    ad = ad / (np.linalg.norm(ad, axis=-1, keepdims=True) + 1e-6)
    fs_new = ad * np.linalg.norm(fs, axis=-1, keepdims=True)
    knots_rel = np.concatenate(
        [knots_rel[:, :, :1], fs_new, knots_rel[:, :, 2:]], axis=2
    )
    last = knots_rel[:, :, -1:, :] + intent[:, None, None, :] * 0.5
    knots_rel = np.concatenate([knots_rel[:, :, :-1], last], axis=2)

    N = B * M
    combined = np.concatenate(
        [knots_rel, sigmas_raw], axis=-1
    ).reshape(N, K, 4).astype(np.float32)
    bits = (geo[..., 1:5, 4].reshape(N, 4) > 0.0).astype(np.int32)
    pat = bits[:, 0] + 2 * bits[:, 1] + 4 * bits[:, 2] + 8 * bits[:, 3]
    # traj[n] = S[pat[n]] @ combined[n]; grouped by pattern for BLAS
    traj = np.empty((N, MAX_T, 4), np.float32)
    for p in range(16):
        idx = np.nonzero(pat == p)[0]
        if idx.size == 0:
            continue
        xg = combined[idx].reshape(idx.size, K * 4)
        # (n,6,4)->(40,n*4): S[p] (40,6) @ each item's (6,4)
        tg = (_S16[p] @ combined[idx].transpose(1, 0, 2).reshape(K, -1))
        traj[idx] = tg.reshape(MAX_T, idx.size, 4).transpose(1, 0, 2)
    mu_t = np.ascontiguousarray(traj[..., :3].reshape(B, M, MAX_T, 3))
    sigma_t = np.ascontiguousarray(traj[..., 3:].reshape(B, M, MAX_T, 1))
    return (
        logits.astype(np.float32),
        mu_t.astype(np.float32),
        sigma_t.astype(np.float32),
        knot_mask.astype(np.float32),
        knot_steps.astype(np.float32),
    )


def _run_device(in_maps, trace=False):
    from concourse.bass_utils import run_bass_kernel_spmd

    if "nc" not in _CACHE:
        _CACHE["nc"] = _build_bass()
    try:
        res = run_bass_kernel_spmd(
            _CACHE["nc"], in_maps, core_ids=list(range(NCORES)), trace=trace
        )
    except ModuleNotFoundError:
        # no NTFF profile hook in this container — run untraced
        res = run_bass_kernel_spmd(
            _CACHE["nc"], in_maps, core_ids=list(range(NCORES)), trace=False
        )
    raw = np.concatenate(
        [res.results[c]["raw"].reshape(BS, OUTC) for c in range(NCORES)], axis=0
    )
    return raw, res.exec_time_ns


def kernel(
    latent_situation,
    intent,
    previous_velocity,
    W1, b1, ln_g, ln_b, W2, b2, Wp, bp,
    _trace=False,
):
    latent_situation = np.asarray(latent_situation, np.float32)
    intent = np.asarray(intent, np.float32)
    previous_velocity = np.asarray(previous_velocity, np.float32)
    W2 = np.asarray(W2, np.float32)
    Wp = np.asarray(Wp, np.float32)
    b2 = np.asarray(b2, np.float32)
    bp = np.asarray(bp, np.float32)
    _CACHE["w2m"] = np.ascontiguousarray(W2.reshape(2, 128, H).transpose(1, 0, 2))
    _CACHE["wpm"] = np.ascontiguousarray(Wp.reshape(2, 128, OUTC).transpose(1, 0, 2))
    in_maps = _prep_inputs(
        latent_situation, intent,
        np.asarray(W1, np.float32), np.asarray(b1, np.float32),
        np.asarray(ln_g, np.float32), np.asarray(ln_b, np.float32),
    )
    raw, exec_ns = _run_device(in_maps, trace=_trace)
    raw = raw + (b2 @ Wp + bp)[None, :]  # biases folded out of the device kernel
    out = _postprocess(raw, intent, previous_velocity)
    if _trace:
        return out, exec_ns
    return out


# revision 10
# speedup vs baseline: 1.0624x; 1.0624x over previous
"""Trainium2 Bass kernel for nn_Actor (ragged spline actor head).

Strategy: pure data-parallel over batch B=16384 across 8 NeuronCores
(2048 rows/core). The device kernel computes the MLP trunk:
  x=[latent,intent,1] @ W1p -> LayerNorm -> affine -> ELU -> @(W2@Wp)
(W2@Wp fused on host: no nonlinearity between layers 2 and 3)
producing raw (B, 248) (biases b2/bp are folded into a constant vector
added on host: raw += b2@Wp + bp). Activations are shipped
pre-transposed (feature-on-partition) so layer-1 needs no on-device
transpose; layers 2/3 transpose activations on the TensorEngine.

fp32 matmuls lower to a self-loading LDWEIGHTS whose ISA wait table
holds a single sync wait, so every SBUF operand feeding the PE is
produced by a final ACT (ScalarE) instruction and every PSUM slot's
last consumer is ACT — all PE waits then collapse onto the one ACT
semaphore.

The ragged Catmull-Rom spline stage is LINEAR in the knots for each of
the 16 possible middle-knot mask patterns (boundary knots always
active), so it reduces to traj[n] = S[p(n)] @ combined[n] with a
precomputed (16, 40, 6) selection/basis table S; the cheap geometry +
selection runs vectorized on host.
"""

import sys

sys.path.insert(0, "/opt/trn_rl_repo")

import numpy as np

B, M, K, RES = 16384, 8, 6, 40
LATENT, INTENT, H = 1024, 3, 256
OUTC = M * (1 + K * 5)  # 248
NCORES = 8
BS = B // NCORES  # 2048 rows per core
NT = BS // 128  # 16 row-tiles per core
FC = 9  # feature chunks: 1024 latent + 3 intent + 1 ones + pad -> 1152
FPAD = FC * 128
PPS = {k: max(4, int(np.ceil(RES / (k - 1)))) for k in range(2, K + 1)}
MAX_T = 40

_CACHE = {}


# ---------------------------------------------------------------- spline table
def _catmull_basis(k):
    """C_k (MAX_T, k): linear map active-knots -> trajectory, matching
    reference._catmull_fixed_k (reflected end padding, last-point pad)."""
    pts = np.eye(k, dtype=np.float64)[:, :, None]  # (k_basis, k, 1)
    p0 = 2 * pts[:, :1] - pts[:, 1:2]
    pn = 2 * pts[:, -1:] - pts[:, -2:-1]
    pad = np.concatenate([p0, pts, pn], axis=1)  # (k, k+2, 1)
    t = np.linspace(0.0, 1.0, PPS[k])[:-1][None, None, :, None]
    t2, t3 = t * t, t * t * t
    P0, P1 = pad[:, 0 : k - 1, None], pad[:, 1:k, None]
    P2, P3 = pad[:, 2 : k + 1, None], pad[:, 3 : k + 2, None]
    seg = 0.5 * (
        2 * P1
        + (-P0 + P2) * t
        + (2 * P0 - 5 * P1 + 4 * P2 - P3) * t2
        + (-P0 + 3 * P1 - 3 * P2 + P3) * t3
    )  # (k, k-1, pps-1, 1)
    traj = np.concatenate([seg.reshape(k, -1, 1), pts[:, -1:]], axis=1)
    padn = MAX_T - traj.shape[1]
    if padn > 0:
        traj = np.concatenate([traj, np.repeat(traj[:, -1:], padn, axis=1)], axis=1)
    return traj[..., 0].T  # (MAX_T, k)


def _sel_table():
    """S (16, MAX_T, 6): traj = S[pattern] @ knots6 where pattern bits are
    the activity of middle knots 1..4 (knot 0 and 5 always active)."""
    S = np.zeros((16, MAX_T, K), dtype=np.float64)
    for p in range(16):
        act = [0] + [i + 1 for i in range(4) if (p >> i) & 1] + [5]
        C = _catmull_basis(len(act))
        for i, j in enumerate(act):
            S[p, :, j] = C[:, i]
    return S.astype(np.float32)


_S16 = _sel_table()


# ---------------------------------------------------------------- bass builder
def _build_bass():
    from concourse import bacc
    import concourse.mybir as mybir
    from concourse.tile import TileContext
    from concourse.masks import make_identity

    AF = mybir.ActivationFunctionType
    AL = mybir.AluOpType
    f32 = mybir.dt.float32

    nc = bacc.Bacc()
    xt = nc.dram_tensor("xt", [NT, 128, FC, 128], f32, kind="ExternalInput")
    w1 = nc.dram_tensor("w1", [128, FC, H], f32, kind="ExternalInput")
    w2 = nc.dram_tensor("w2", [128, 2, OUTC], f32, kind="ExternalInput")
    gb = nc.dram_tensor("gb", [128, H], f32, kind="ExternalInput")
    bb = nc.dram_tensor("bb", [128, H], f32, kind="ExternalInput")
    raw = nc.dram_tensor("raw", [NT, 128, OUTC], f32, kind="ExternalOutput")

    with TileContext(nc) as tc:
        with (
            tc.tile_pool(name="consts", bufs=1) as cp,
            tc.tile_pool(name="work", bufs=3) as wk,
            tc.tile_pool(name="psum", bufs=2, space="PSUM") as pp,
            tc.tile_pool(name="psumt", bufs=2, space="PSUM") as pt,
        ):
            # Weights: DMA to a staging tile, then ACT-copy to the tile the
            # PE reads, so PE waits collapse onto the ACT semaphore.
            w1s = cp.tile([128, FC, H], f32)
            nc.sync.dma_start(w1s[:], w1[:])
            w2s = cp.tile([128, 2, OUTC], f32)
            nc.sync.dma_start(w2s[:], w2[:])
            ident = cp.tile([128, 128], f32)
            make_identity(nc, ident[:])

            gbs = cp.tile([128, H], f32)
            nc.sync.dma_start(gbs[:], gb[:])
            bbs = cp.tile([128, H], f32)
            nc.sync.dma_start(bbs[:], bb[:])
            neg1 = cp.tile([128, 1], f32)
            nc.vector.memset(neg1[:], -1.0)

            for t in range(NT):
                xsb = wk.tile([128, FC, 128], f32, tag="xsb")
                nc.sync.dma_start(xsb[:], xt[t])

                ph = pp.tile([128, H], f32, tag="ph")
                for c in range(FC):
                    nc.tensor.matmul(
                        ph[:], xsb[:, c, :], w1s[:, c, :],
                        start=(c == 0), stop=(c == FC - 1),
                    )

                # LayerNorm over H (free dim): stats via ACT accumulate.
                # ph's readers are both ACT so the next ph matmul has 1 wait.
                hsb = wk.tile([128, H], f32, tag="hsb")
                s1 = wk.tile([128, 1], f32, tag="s1")
                nc.scalar.activation(hsb[:], ph[:], AF.Identity, accum_out=s1[:])
                h2 = wk.tile([128, H], f32, tag="h2")
                s2 = wk.tile([128, 1], f32, tag="s2")
                nc.scalar.activation(h2[:], ph[:], AF.Square, accum_out=s2[:])
                mu = wk.tile([128, 1], f32, tag="mu")
                nc.scalar.mul(mu[:], s1[:], 1.0 / H)
                ex2 = wk.tile([128, 1], f32, tag="ex2")
                nc.scalar.mul(ex2[:], s2[:], 1.0 / H)
                mu2 = wk.tile([128, 1], f32, tag="mu2")
                nc.vector.tensor_mul(mu2[:], mu[:], mu[:])
                var = wk.tile([128, 1], f32, tag="var")
                nc.vector.tensor_sub(var[:], ex2[:], mu2[:])
                nc.vector.tensor_scalar_add(var[:], var[:], 1e-5)
                std = wk.tile([128, 1], f32, tag="std")
                nc.scalar.activation(std[:], var[:], AF.Sqrt)
                inv = wk.tile([128, 1], f32, tag="inv")
                nc.vector.reciprocal(inv[:], std[:])
                hn = wk.tile([128, H], f32, tag="hn")
                nc.vector.tensor_scalar(
                    hn[:], hsb[:], mu[:], inv[:], AL.subtract, AL.mult
                )
                nc.vector.tensor_mul(hn[:], hn[:], gbs[:])
                nc.vector.tensor_add(hn[:], hn[:], bbs[:])

                # ELU = max(x,0) + exp(min(x,0)) - 1; final writer is ACT
                tmin = wk.tile([128, H], f32, tag="tmin")
                nc.vector.tensor_scalar_min(tmin[:], hn[:], 0.0)
                ee = wk.tile([128, H], f32, tag="ee")
                nc.scalar.activation(ee[:], tmin[:], AF.Exp)
                q = wk.tile([128, H], f32, tag="q")
                nc.vector.tensor_scalar_max(q[:], hn[:], 0.0)
                nc.vector.tensor_add(q[:], q[:], ee[:])
                el = wk.tile([128, H], f32, tag="el")
                nc.scalar.activation(el[:], q[:], AF.Identity, bias=neg1[:])

                # transpose elu -> feature-on-partition
                eT = wk.tile([128, 2, 128], f32, tag="eT")
                for c in range(2):
                    ptt = pt.tile([128, 128], f32, tag="tp")
                    nc.tensor.transpose(
                        ptt[:], el[:, c * 128 : (c + 1) * 128], ident[:]
                    )
                    nc.scalar.copy(eT[:, c, :], ptt[:])

                pr = pp.tile([128, OUTC], f32, tag="pr")
                for c in range(2):
                    nc.tensor.matmul(
                        pr[:], eT[:, c, :], w2s[:, c, :],
                        start=(c == 0), stop=(c == 1),
                    )
                rsb = wk.tile([128, OUTC], f32, tag="rsb")
                nc.scalar.copy(rsb[:], pr[:])
                nc.sync.dma_start(raw[t], rsb[:])
    nc.compile()
    return nc


# ---------------------------------------------------------------- host helpers
def _prep_inputs(latent, intent, W1, b1, ln_g, ln_b):
    """Build per-core in_maps (weights replicated, x sharded+pretransposed)."""
    xp = np.zeros((B, FPAD), dtype=np.float32)
    xp[:, :LATENT] = latent
    xp[:, LATENT : LATENT + INTENT] = intent
    xp[:, LATENT + INTENT] = 1.0  # ones column -> b1 via matmul

    W1p = np.zeros((FPAD, H), dtype=np.float32)
    W1p[: LATENT + INTENT] = W1
    W1p[LATENT + INTENT] = b1
    w1m = np.ascontiguousarray(W1p.reshape(FC, 128, H).transpose(1, 0, 2))
    gbm = np.ascontiguousarray(np.broadcast_to(ln_g, (128, H)).astype(np.float32))
    bbm = np.ascontiguousarray(np.broadcast_to(ln_b, (128, H)).astype(np.float32))

    in_maps = []
    for c in range(NCORES):
        xs = xp[c * BS : (c + 1) * BS]  # (2048, 1152)
        # device tile layout: [t, feat_in_chunk, chunk, batch_col]
        xtc = np.ascontiguousarray(
            xs.reshape(NT, 128, FC, 128).transpose(0, 3, 2, 1)
        )
        in_maps.append(
            {
                "xt": xtc,
                "w1": w1m,
                "w2": _CACHE["w2m"],
                "gb": gbm,
                "bb": bbm,
            }
        )
    return in_maps


def _postprocess(raw, intent, previous_velocity):
    """Geometry + ragged-spline stage (vectorized numpy), matching reference."""
    raw = raw.reshape(B, M, 1 + K * 5)
    logits = np.ascontiguousarray(raw[:, :, 0])
    geo = raw[:, :, 1:].reshape(B, M, K, 5)
    knot_steps = np.tanh(geo[..., :3]) * 2.0
    sigmas_raw = np.logaddexp(0.0, geo[..., 3:4]).astype(np.float32) + 0.1
    mask_raw = 1.0 / (1.0 + np.exp(-geo[..., 4]))
    ones = np.ones((B, M, 1), mask_raw.dtype)
    knot_mask = np.concatenate([ones, mask_raw[:, :, 1:-1], ones], axis=-1)
    knots_rel = np.cumsum(knot_steps, axis=2)
    knots_rel = knots_rel - knots_rel[:, :, 0:1, :]
    pv = previous_velocity
    pvn = pv / (np.linalg.norm(pv, axis=-1, keepdims=True) + 1e-6)
    fs = knots_rel[:, :, 1:2, :]
    fsn = fs / (np.linalg.norm(fs, axis=-1, keepdims=True) + 1e-6)
    ad = 0.8 * pvn[:, None, None, :] + 0.2 * fsn
    ad = ad / (np.linalg.norm(ad, axis=-1, keepdims=True) + 1e-6)
    fs_new = ad * np.linalg.norm(fs, axis=-1, keepdims=True)
    knots_rel = np.concatenate(
        [knots_rel[:, :, :1], fs_new, knots_rel[:, :, 2:]], axis=2
    )
    last = knots_rel[:, :, -1:, :] + intent[:, None, None, :] * 0.5
    knots_rel = np.concatenate([knots_rel[:, :, :-1], last], axis=2)

    N = B * M
    combined = np.concatenate(
        [knots_rel, sigmas_raw], axis=-1
    ).reshape(N, K, 4).astype(np.float32)
    bits = (geo[..., 1:5, 4].reshape(N, 4) > 0.0).astype(np.int32)
    pat = bits[:, 0] + 2 * bits[:, 1] + 4 * bits[:, 2] + 8 * bits[:, 3]
    # traj[n] = S[pat[n]] @ combined[n]; grouped by pattern for BLAS
    traj = np.empty((N, MAX_T, 4), np.float32)
    for p in range(16):
        idx = np.nonzero(pat == p)[0]
        if idx.size == 0:
            continue
        xg = combined[idx].reshape(idx.size, K * 4)
        # (n,6,4)->(40,n*4): S[p] (40,6) @ each item's (6,4)
        tg = (_S16[p] @ combined[idx].transpose(1, 0, 2).reshape(K, -1))
        traj[idx] = tg.reshape(MAX_T, idx.size, 4).transpose(1, 0, 2)
    mu_t = np.ascontiguousarray(traj[..., :3].reshape(B, M, MAX_T, 3))
    sigma_t = np.ascontiguousarray(traj[..., 3:].reshape(B, M, MAX_T, 1))
    return (
        logits.astype(np.float32),
        mu_t.astype(np.float32),
        sigma_t.astype(np.float32),
        knot_mask.astype(np.float32),
        knot_steps.astype(np.float32),
    )


def _run_device(in_maps, trace=False):
    from concourse.bass_utils import run_bass_kernel_spmd

    if "nc" not in _CACHE:
        _CACHE["nc"] = _build_bass()
    try:
        res = run_bass_kernel_spmd(
            _CACHE["nc"], in_maps, core_ids=list(range(NCORES)), trace=trace
        )
    except ModuleNotFoundError:
        # no NTFF profile hook in this container — run untraced
        res = run_bass_kernel_spmd(
            _CACHE["nc"], in_maps, core_ids=list(range(NCORES)), trace=False
        )
    raw = np.concatenate(
        [res.results[c]["raw"].reshape(BS, OUTC) for c in range(NCORES)], axis=0
    )
    return raw, res.exec_time_ns


def kernel(
    latent_situation,
    intent,
    previous_velocity,
    W1, b1, ln_g, ln_b, W2, b2, Wp, bp,
    _trace=False,
):
    latent_situation = np.asarray(latent_situation, np.float32)
    intent = np.asarray(intent, np.float32)
    previous_velocity = np.asarray(previous_velocity, np.float32)
    W2 = np.asarray(W2, np.float32)
    Wp = np.asarray(Wp, np.float32)
    b2 = np.asarray(b2, np.float32)
    bp = np.asarray(bp, np.float32)
    W2p = (W2.astype(np.float64) @ Wp.astype(np.float64)).astype(np.float32)
    _CACHE["w2m"] = np.ascontiguousarray(W2p.reshape(2, 128, OUTC).transpose(1, 0, 2))
    in_maps = _prep_inputs(
        latent_situation, intent,
        np.asarray(W1, np.float32), np.asarray(b1, np.float32),
        np.asarray(ln_g, np.float32), np.asarray(ln_b, np.float32),
    )
    raw, exec_ns = _run_device(in_maps, trace=_trace)
    raw = raw + (b2 @ Wp + bp)[None, :]  # biases folded out of the device kernel
    out = _postprocess(raw, intent, previous_velocity)
    if _trace:
        return out, exec_ns
    return out
